# revision 61
# baseline (speedup 1.0000x reference)
"""Trainium2 Bass kernel for MultiHeadAttention (B=4, S=2048, D=1024, H=16).

Sharding: 8 cores = 4 batches x 2 sequence-halves, no collectives. Each
core computes full k/v projections for its batch and q/attention/
out-proj/LayerNorm for its half of the sequence. A host-side column
roll of x^T makes the program identical across cores (softmax over
keys is permutation-invariant): the core's q rows are always columns
[0, SQ) of its x^T.

Device program (per core):

Projections run as residual-fp8 DoubleRow matmuls: host splits x and
W (W pre-scaled by 32 so e4m3 normals cover it) into e4m3 hi+lo
planes; W.x ~= (W_hi+W_lo).x_hi + W_hi.x_lo (the dropped lo.lo term
is ~0.1%). Each K=128 bf16 matmul pair becomes a K=256 DoubleRow
fp8 matmul at half cost, 0.75x total. PSUM holds 32x the result; the
PSUM->SBUF bias-add copy folds in the 1/32 (v keeps the 32x scale,
cancelled by a 32-valued "ones" column in the softmax denominator).

Attention per head pair m (heads 2m/2m+1 on partition halves of
kt/qt), per q-chunk n:
  scT[j]  = kT_h[:, j-tile] . qT_h    both heads -> one 2-bank PSUM
  eT[j]   = exp(scT[j]/8)             one wide ScalarE op, bf16
  av[qs] += eT[j]_h[:, qs-tile].T @ v_aug[j]_h   [128q, 65] PSUM:
            e is the stationary operand so the moving free dim is 65
            (=FLOP-ideal; the old orientation re-streamed q per key
            tile at 2x cost). Row 64 = denominator via the 32-column.
  aotB    = av[:, 0:64] / av[:, 64]   per-partition scalar divide
  aotT    = PE-transpose(aotB)        -> aot [d, q] for the out-proj
out = aotT.T @ Wo.T + bo; LayerNorm (rstd via Ln+Exp so ScalarE stays
on one activation table set) -> DRAM.
"""

import os
import sys
from contextlib import ExitStack

for _p in ("/opt/trn_rl_repo", "/root/.axon_site/_ro/trn_rl_repo"):
    if _p not in sys.path and os.path.isdir(_p):
        sys.path.insert(0, _p)

# The kernel executes through the axon jax platform; a cpu-pinned
# JAX_PLATFORMS (used for running references) would hide the NeuronCores.
if "jax" not in sys.modules and "axon" not in os.environ.get(
        "JAX_PLATFORMS", "axon"):
    os.environ.pop("JAX_PLATFORMS")

import ml_dtypes
import numpy as np

import concourse.bacc as bacc
import concourse.mybir as mybir
import concourse.tile as tile
from concourse import library_config
from concourse.bass_utils import run_bass_kernel_spmd

BF16 = mybir.dt.bfloat16
F32 = mybir.dt.float32
F8 = mybir.dt.float8e4
AF = mybir.ActivationFunctionType
ALU = mybir.AluOpType

HD = 64      # head dim
WSCALE = 32.0  # host pre-scale on projection weights (fp8 range)


def build_bass(S, SQ, D, H, dtype=BF16, debug=False):
    """Build the per-core Bass program. S: kv seq len, SQ: q rows handled
    by this core, D: embed dim, H: total heads."""
    assert D == H * HD
    P = 128
    ET = D // P           # contraction tiles; also head-pair count
    QC = min(512, SQ)     # q free-dim chunk
    QN = SQ // QC
    KC = min(512, S)
    KN = S // KC
    VC = min(512, D)      # v/out-proj d chunk
    VN = D // VC
    HPC = VC // HD        # heads per v chunk
    SJ = S // P           # key tiles
    TQ = SQ // P          # q row tiles
    QS = QC // P          # q sub-tiles per chunk (AV stationary slices)

    nc = bacc.Bacc("TRN2", debug=False)

    x8 = nc.dram_tensor("x8", [P, ET, 2, S], F8, kind="ExternalInput").ap()
    ws = {}
    for w in ("wq8", "wk8"):   # per head-pair m: [m, p, kt*2*d]
        ws[w] = nc.dram_tensor(w, [ET, P, ET * 2 * P], F8,
                               kind="ExternalInput").ap()
    for w in ("wqh8", "wkh8"):  # hi-only copies for the cross term
        ws[w] = nc.dram_tensor(w, [ET, P, ET * P], F8,
                               kind="ExternalInput").ap()
    ws["wv8"] = nc.dram_tensor("wv8", [P, ET * 2 * D], F8,
                               kind="ExternalInput").ap()
    ws["wvh8"] = nc.dram_tensor("wvh8", [P, ET * D], F8,
                                kind="ExternalInput").ap()
    ws["wo"] = nc.dram_tensor("wo", [D, D], dtype, kind="ExternalInput").ap()
    ws["ident"] = nc.dram_tensor("ident", [P, P], dtype,
                                 kind="ExternalInput").ap()
    bs = {
        b: nc.dram_tensor(b, [D], F32, kind="ExternalInput").ap()
        for b in ("bq", "bk")
    }
    bs.update({
        b: nc.dram_tensor(b, [D], dtype, kind="ExternalInput").ap()
        for b in ("bv", "bo")
    })
    out = nc.dram_tensor("out", [SQ, D], F32, kind="ExternalOutput").ap()
    dbg = {}
    if debug:
        for nm, shp in (("dq0", [P, SQ]), ("dk0", [P, S]),
                        ("dv", [P, SJ * H * (HD + 1)]),
                        ("daot", [P, ET * SQ]),
                        ("det0", [P, 2 * QC]), ("dab0", [P, P])):
            dbg[nm] = nc.dram_tensor(nm, shp, BF16, kind="ExternalOutput").ap()

    with tile.TileContext(nc) as tc, ExitStack() as ctx:
        singles = ctx.enter_context(tc.tile_pool(name="singles", bufs=1))
        xp = ctx.enter_context(tc.tile_pool(name="xp", bufs=1))
        qtp = ctx.enter_context(tc.tile_pool(name="qtp", bufs=3))
        ktp = ctx.enter_context(tc.tile_pool(name="ktp", bufs=3))
        qkv = ctx.enter_context(tc.tile_pool(name="qkv", bufs=1))
        wqk = ctx.enter_context(tc.tile_pool(name="wqk", bufs=2))
        wvp = ctx.enter_context(tc.tile_pool(name="wvp", bufs=1))
        wop = ctx.enter_context(tc.tile_pool(name="wop", bufs=1))
        expp = ctx.enter_context(tc.tile_pool(name="expp", bufs=18))
        aotbp = ctx.enter_context(tc.tile_pool(name="aotbp", bufs=4))
        outp = ctx.enter_context(tc.tile_pool(name="outp", bufs=2))
        lnp = ctx.enter_context(tc.tile_pool(name="lnp", bufs=5))
        mmp = ctx.enter_context(tc.tile_pool(name="mm", bufs=2, space="PSUM"))
        scp = ctx.enter_context(tc.tile_pool(name="scp", bufs=2, space="PSUM"))
        avp = ctx.enter_context(tc.tile_pool(name="avp", bufs=1, space="PSUM"))

        nc.gpsimd.load_library(library_config.proxy)

        vt = qkv.tile([P, SJ, H, HD + 1], dtype, tag="vt")
        aot = qkv.tile([P, ET, SQ], dtype, tag="aot")
        qts, kts = {}, {}

        def load_wqk(m, eng=None):
            eng = eng or nc.gpsimd
            tiles = []
            for w, wh in (("wq8", "wqh8"), ("wk8", "wkh8")):
                wm = wqk.tile([P, ET, 2, P], F8, tag=w, name=w)
                eng.dma_start(
                    wm, ws[w][m].rearrange("p (t i d) -> p t i d", i=2, d=P))
                whm = wqk.tile([P, ET, P], F8, tag=wh, name=wh)
                eng.dma_start(
                    whm, ws[wh][m].rearrange("p (t d) -> p t d", d=P))
                tiles += [wm, whm]
            return tiles

        # --- x^T fp8 planes (hi, hi, lo), alternating HWDGE queues so the
        # projections aren't DMA-starved at startup
        pre0 = load_wqk(0, nc.scalar)
        pre1 = load_wqk(1, nc.scalar)
        x8t = xp.tile([P, ET, 2, S], F8, tag="x8t")
        QTR = S // 4
        qs_engs = (nc.sync, nc.gpsimd, nc.scalar)
        di = 0
        for c in range(4):
            for k in range(ET):
                eng = qs_engs[di % 3]
                di += 1
                eng.dma_start(
                    x8t[:, k, :, c * QTR:(c + 1) * QTR],
                    x8[:, k, :, c * QTR:(c + 1) * QTR])

        # --- constants ---
        bqk = singles.tile([P, 2 * ET], F32, tag="bqk")
        nc.sync.dma_start(bqk[:, :ET], bs["bq"].rearrange("(t p) -> p t", p=P))
        nc.sync.dma_start(bqk[:, ET:], bs["bk"].rearrange("(t p) -> p t", p=P))
        ident = singles.tile([P, P], dtype, tag="ident")
        nc.sync.dma_start(ident, ws["ident"])
        # free-dim bias rows, physically replicated across partitions
        brow = {}
        for b in ("bv", "bo"):
            t = singles.tile([P, D], dtype, tag=b)
            nc.sync.dma_start(t[0:1, :], bs[b][None, :])
            nc.gpsimd.partition_broadcast(t, t[0:1, :])
            brow[b] = t
        # denominator column: v carries a 32x scale, so the "ones" are 32
        nc.vector.memset(vt[:, :, :, HD:HD + 1], WSCALE)

        DR = mybir.MatmulPerfMode.DoubleRow

        urgent_q = []  # work needed promptly (v0 chunks, transposes):
                       # one item per attention slot, ahead of paced fill
        fill_q = []    # (cost_ns, closure) projection work, drained inside
                       # the attention j-loops at a credit-based rate sized
                       # to last through ALL the blocks
        slots_left = [0]
        fill_state = [0.0, 0.0]  # total queued cost, accumulated credit

        TOT_SLOTS = [0]

        def drain_fill():
            if urgent_q:
                urgent_q.pop(0)()
            if not fill_q:
                return
            cur = TOT_SLOTS[0] - slots_left[0]
            fill_state[1] += fill_state[0] / max(slots_left[0], 1)
            n = 0
            while fill_q and n < 3:
                c, fn, dl = fill_q[0]
                # pop on accumulated credit, or when the consumer block is
                # near (deadline keeps pool-rotation overwrites ordered
                # after the delayed chunk reads)
                if fill_state[1] >= c or dl - 16 <= cur:
                    fill_q.pop(0)
                    fill_state[0] -= c
                    fill_state[1] = max(0.0, fill_state[1] - c)
                    fn()
                    n += 1
                else:
                    break

        def queue_fill(cost, fn, deadline=10**9):
            fill_q.append((cost, fn, deadline))
            fill_state[0] += cost

        def qk_proj(m, pre=None, defer=False):
            dl = m * 2 * SJ
            """q and k projections for head-pair m (residual-fp8 DR)."""
            wqm, wqhm, wkm, wkhm = pre if pre is not None else load_wqk(m)
            qt = qtp.tile([P, SQ], dtype, tag="qt", name="qt")
            kt = ktp.tile([P, S], dtype, tag="kt", name="kt")
            qts[m], kts[m] = qt, kt
            order = [(0, 0), (1, 0), (1, 1), (0, 1), (1, 2), (1, 3)]
            specs = ((qt, wqm, wqhm, QN, QC, m),
                     (kt, wkm, wkhm, KN, KC, ET + m))
            for which, n in order:
                dst, wm, whm, nch, cc, bofs = specs[which]
                if True:
                    def chunk(dst=dst, wm=wm, whm=whm, n=n, cc=cc, bofs=bofs):
                        cols = slice(n * cc, (n + 1) * cc)
                        ps = mmp.tile([P, 512], F32, tag="mm", name="ps")[:, :cc]
                        for k in range(ET):
                            nc.tensor.matmul(
                                ps, wm[:, k], x8t[:, k, 0:2, cols],
                                start=(k == 0), stop=False, perf_mode=DR)
                        for kp in range(ET // 2):
                            nc.tensor.matmul(
                                ps, whm[:, 2 * kp:2 * kp + 2, :],
                                x8t[:, 2 * kp:2 * kp + 2, 0, cols],
                                start=False, stop=(kp == ET // 2 - 1),
                                perf_mode=DR)
                        nc.vector.tensor_scalar(
                            dst[:, cols], ps, 1.0 / WSCALE,
                            bqk[:, bofs:bofs + 1], ALU.mult, ALU.add)
                    if defer:
                        queue_fill(1300, chunk, dl)
                    else:
                        chunk()

        def load_wv(n, eng=None):
            eng = eng or nc.gpsimd
            wvn = wvp.tile([P, ET, 2, VC], F8, tag="wv", name="wvn")
            eng.dma_start(
                wvn,
                ws["wv8"].rearrange("p (t i d) -> p t i d", i=2, d=D)
                [:, :, :, n * VC:(n + 1) * VC])
            wvhn = wvp.tile([P, ET, VC], F8, tag="wvh", name="wvhn")
            eng.dma_start(
                wvhn,
                ws["wvh8"].rearrange("p (t d) -> p t d", d=D)
                [:, :, n * VC:(n + 1) * VC])
            return wvn, wvhn

        def v_block(n, wvn, wvhn, j):
            """v projection d-chunk n, s-tile j (keeps 32x scale)."""
            jcols = slice(j * P, (j + 1) * P)
            ps = mmp.tile([P, 512], F32, tag="mm", name="ps")[:, :VC]
            for k in range(ET):
                nc.tensor.matmul(
                    ps, x8t[:, k, 0:2, jcols], wvn[:, k],
                    start=(k == 0), stop=False, perf_mode=DR)
            for kp in range(ET // 2):
                nc.tensor.matmul(
                    ps, x8t[:, 2 * kp:2 * kp + 2, 0, jcols],
                    wvhn[:, 2 * kp:2 * kp + 2, :],
                    start=False, stop=(kp == ET // 2 - 1), perf_mode=DR)
            nc.vector.tensor_tensor(
                vt[:, j, n * HPC:(n + 1) * HPC, 0:HD],
                ps.rearrange("p (h d) -> p h d", d=HD),
                brow["bv"][:, n * VC:(n + 1) * VC].rearrange(
                    "p (h d) -> p h d", d=HD),
                ALU.add,
            )

        def v_proj(n, wv, defer=False):
            for j in range(SJ):
                if defer == "urgent":
                    urgent_q.append(
                        lambda n=n, wv=wv, j=j: v_block(n, wv[0], wv[1], j))
                elif defer:
                    queue_fill(
                        1300,
                        lambda n=n, wv=wv, j=j: v_block(n, wv[0], wv[1], j),
                        (H // HPC // 2) * 2 * SJ)
                else:
                    v_block(n, wv[0], wv[1], j)

        def att_exp(m, n, j):
            """score pair + exp for (head pair m, q-chunk n, k-tile j)."""
            qt, kt = qts[m], kts[m]
            sc = scp.tile([P, 2, 512], F32, tag="sc", name="sc")
            nc.tensor.matmul(
                sc[:, 0, :QC],
                kt[0:HD, j * P:(j + 1) * P],
                qt[0:HD, n * QC:(n + 1) * QC],
            )
            nc.tensor.matmul(
                sc[:, 1, :QC],
                kt[HD:P, j * P:(j + 1) * P],
                qt[HD:P, n * QC:(n + 1) * QC],
            )
            et = expp.tile([P, 2, QC], dtype, tag="exp", name="et")
            nc.scalar.activation(et, sc[:, :, :QC], AF.Exp, scale=0.125)
            if debug and m == 0 and n == 0 and j == 0:
                nc.sync.dma_start(dbg["det0"], et.rearrange("p a b -> p (a b)"))
            return et

        def av_mms(m, et, j, av4):
            # av4 slices padded to 512B so no matmul output crosses a
            # 2KB PSUM bank boundary. start=True clears the WHOLE bank,
            # so only the first matmul touching each bank carries it; the
            # other slices' first write lands on cleared has_written bits
            # and overwrites.
            for qs in range(QS):
                for h in range(2):
                    k = 2 * qs + h
                    nc.tensor.matmul(
                        av4[:, k, 0:HD + 1],
                        et[:, h, qs * P:(qs + 1) * P],
                        vt[:, j, 2 * m + h, :],
                        start=(j == 0 and k % 4 == 0), stop=(j == SJ - 1),
                        skip_group_check=True,
                    )

        tail_work = []  # previous block's last AV group + normalize

        def block_tail(m, n, pend, av4):
            """Final AV matmuls + per-head normalize of a block. Deferred
            until after the NEXT block's first scores so the PE never
            stalls on the block's last exps."""
            for j, et in pend:
                av_mms(m, et, j, av4)
            for qs in range(QS):
                t = n * QS + qs
                ab = aotbp.tile([P, 2 * HD], dtype, tag="ab", name="ab")
                rcp = aotbp.tile([P, 2], F32, tag="rcp", name="rcp")
                for h in range(2):
                    k = 2 * qs + h
                    nc.vector.reciprocal(
                        rcp[:, h:h + 1], av4[:, k, HD:HD + 1])
                    nc.vector.tensor_scalar(
                        ab[:, h * HD:(h + 1) * HD], av4[:, k, 0:HD],
                        rcp[:, h:h + 1], None, ALU.mult)

                if debug and m == 0 and n == 0 and qs == 0:
                    nc.sync.dma_start(dbg["dab0"], ab)

                def finish(m=m, t=t, ab=ab):
                    tr = mmp.tile([P, P], dtype, tag="mm", name="tr")
                    nc.tensor.transpose(tr, ab, ident)
                    nc.vector.tensor_copy(aot[:, m, t * P:(t + 1) * P], tr)
                # run in upcoming slots so the PE transpose never queues
                # ahead of scores while its input is still pending
                urgent_q.append(finish)

        def attention(m, n):
            """q-chunk n of head pair m (heads 2m, 2m+1). The AV matmuls
            trail one key-tile behind their exp so they never head-of-line
            block PE's in-order SEQ; one projection chunk is drained from
            fill_q per key-tile to keep PE busy under the ScalarE exps."""
            av4 = avp.tile([P, 2 * QS, P, ], F32, tag="av", name="av")
            pend = []
            for j in range(SJ):
                et = att_exp(m, n, j)
                if j == 14 and tail_work:
                    tail_work.pop(0)()
                else:
                    drain_fill()
                slots_left[0] -= 1
                # AV trails its exp by two key-tiles so exp jitter never
                # head-of-line blocks PE's in-order SEQ
                if len(pend) >= 14:
                    pj, pet = pend.pop(0)
                    av_mms(m, pet, pj, av4)
                pend.append((j, et))
            tail_work.append(lambda: block_tail(m, n, pend, av4))

        ln_pend = {}

        def out_ln(t, part="all"):
            """Out-projection + LayerNorm for q row tile t. part="front"
            stops before the ScalarE rstd (so overlapping out_lns don't
            interleave table sets with the attention exps); "back" finishes."""
            if part == "back":
                ot, scr, mv, rstd = ln_pend.pop(t)
                nc.scalar.activation(rstd, mv[:, 1:2], AF.Abs_reciprocal_sqrt,
                                     bias=eps)
                nc.vector.tensor_scalar(
                    ot, ot, mv[:, 0:1], rstd, ALU.subtract, ALU.mult)
                nc.sync.dma_start(
                    out.rearrange("(t p) d -> p t d", p=P)[:, t, :], ot)
                return
            FSUB = min(512, D)
            NSUB = D // FSUB
            ot = outp.tile([P, D], F32, tag="ot", name="ot")
            for nn in range(VN):
                ps = mmp.tile([P, 512], F32, tag="mm", name="ps")[:, :VC]
                for k in range(ET):
                    nc.tensor.matmul(
                        ps, aot[:, k, t * P:(t + 1) * P],
                        wo[:, k, nn * VC:(nn + 1) * VC],
                        start=(k == 0), stop=(k == ET - 1),
                    )
                nc.vector.tensor_tensor(
                    ot[:, nn * VC:(nn + 1) * VC], ps,
                    brow["bo"][:, nn * VC:(nn + 1) * VC], ALU.add)
            scr = lnp.tile([P, NSUB * 6 + 3], F32, tag="scr", name="scr")
            stats = scr[:, 0:NSUB * 6].rearrange("p (s f) -> p s f", f=6)
            mv = scr[:, NSUB * 6:NSUB * 6 + 2]
            rstd = scr[:, NSUB * 6 + 2:NSUB * 6 + 3]
            otv = ot.rearrange("p (s f) -> p s f", f=FSUB)
            for sbi in range(NSUB):
                nc.vector.bn_stats(stats[:, sbi, :], otv[:, sbi, :])
            nc.vector.bn_aggr(mv, stats)
            if part == "front":
                ln_pend[t] = (ot, scr, mv, rstd)
                return
            nc.scalar.activation(rstd, mv[:, 1:2], AF.Abs_reciprocal_sqrt,
                                 bias=eps)
            nc.vector.tensor_scalar(
                ot, ot, mv[:, 0:1], rstd, ALU.subtract, ALU.mult)
            # device emits the normalized rows; the ln_w/ln_b affine is a
            # pure elementwise epilogue applied host-side on the gathered
            # output (same spirit as the host-side input packing)
            nc.sync.dma_start(
                out.rearrange("(t p) d -> p t d", p=P)[:, t, :], ot)

        # --- emission schedule ---
        wo = wop.tile([P, ET, D], dtype, tag="wo")
        slots_left[0] = ET * QN * SJ
        TOT_SLOTS[0] = slots_left[0]
        qk_proj(0, pre0)
        wv0 = load_wv(0, nc.sync)
        v_proj(0, wv0, defer="urgent")
        qk_proj(1, pre1, defer=True)
        nc.gpsimd.dma_start(wo, ws["wo"].rearrange("(t p) d -> p t d", p=P))
        done_ln = [0]

        def queue_ln(upto, part="all"):
            while done_ln[0] < upto:
                t = done_ln[0]
                urgent_q.append(lambda t=t, part=part: out_ln(t, part))
                done_ln[0] += 1

        for m in range(ET):
            for n in range(QN):
                if m == ET - 1 and n == 1:
                    # first-half out-projs are ready: overlap them with
                    # the final attention block (ScalarE-free front part)
                    queue_ln(QS, "front")
                attention(m, n)
            # queue the remaining projections; the attention j-loops drain
            # them at a rate sized to last through all the blocks
            if m + 2 < ET:
                qk_proj(m + 2, defer=True)
            if m == 0:
                v_proj(1, load_wv(1), defer=True)
        if debug:
            nc.sync.dma_start(dbg["dq0"], qts[0])
            nc.sync.dma_start(dbg["dk0"], kts[0])
            nc.sync.dma_start(
                dbg["dv"], vt.rearrange("p a b c -> p (a b c)"))
        while tail_work:
            tail_work.pop(0)()
        # drain remaining finishers interleaved with the out_lns they gate
        while urgent_q:
            urgent_q.pop(0)()
            queue_ln(min(done_ln[0] + 1, TQ))
        for t in sorted(ln_pend):
            out_ln(t, "back")
        while fill_q:
            fill_q.pop(0)[1]()

        if debug:
            nc.sync.dma_start(dbg["daot"], aot.rearrange("p a b -> p (a b)"))
        # out-proj/LN remainder
        queue_ln(TQ)
        while urgent_q:
            urgent_q.pop(0)()

    nc.compile()
    return nc


# ---------------------------------------------------------------- host side

_CACHE = {}


def _get_nc(S, SQ, D, H):
    key = (S, SQ, D, H)
    if key not in _CACHE:
        _CACHE[key] = build_bass(S, SQ, D, H)
    return _CACHE[key]


F8NP = ml_dtypes.float8_e4m3


def _split8(a):
    """fp32 array -> (hi, lo) e4m3 planes with hi+lo ~= a."""
    hi = np.asarray(a, np.float32).astype(F8NP)
    lo = (np.asarray(a, np.float32) - hi.astype(np.float32)).astype(F8NP)
    return hi, lo


def make_in_maps(x, Wq, bq, Wk, bk, Wv, bv, Wo, bo, ln_w, ln_b, n_cores=8):
    """Shard full inputs into per-core input maps (batch x seq-half)."""
    B, S, D = x.shape
    halves = n_cores // B
    SQ = S // halves
    bf = ml_dtypes.bfloat16
    ET = D // 128
    P = 128

    def pack_qk(W):
        # base[m, p, t, d] = (W*32)[m*128+d, t*128+p]
        hi, lo = _split8(np.asarray(W, np.float32) * WSCALE)

        def arr(z):
            w4 = z.astype(np.float32).T.reshape(ET, P, ET, P)  # [t, p, m, d]
            return np.ascontiguousarray(w4.transpose(2, 1, 0, 3))  # [m,p,t,d]

        ah, al = arr(hi), arr(lo)
        full = np.stack([ah, ah], axis=3)  # [m, p, t, 2, d] (hi dup)
        return (full.reshape(ET, P, ET * 2 * P).astype(F8NP),
                np.ascontiguousarray(al).reshape(ET, P, ET * P).astype(F8NP))

    def pack_v(W):
        hi, lo = _split8(np.asarray(W, np.float32) * WSCALE)

        def arr(z):  # [p, t, dcol] = z.T[t*128+p, dcol]
            return np.ascontiguousarray(
                z.astype(np.float32).T.reshape(ET, P, D).transpose(1, 0, 2))

        ah, al = arr(hi), arr(lo)
        full = np.stack([ah, ah], axis=2)  # [p, t, 2, d] (hi dup)
        return (full.reshape(P, ET * 2 * D).astype(F8NP),
                al.reshape(P, ET * D).astype(F8NP))

    wq8, wqh8 = pack_qk(Wq)
    wk8, wkh8 = pack_qk(Wk)
    wv8, wvh8 = pack_v(Wv)
    common = {
        "wq8": wq8, "wqh8": wqh8, "wk8": wk8, "wkh8": wkh8,
        "wv8": wv8, "wvh8": wvh8,
        "wo": np.ascontiguousarray(np.asarray(Wo).T).astype(bf),
        "ident": np.eye(P, dtype=np.float32).astype(bf),
        "bq": np.asarray(bq, np.float32), "bk": np.asarray(bk, np.float32),
        "bv": (np.asarray(bv, np.float32) * WSCALE).astype(bf),
        "bo": np.asarray(bo, np.float32).astype(bf),
    }
    in_maps = []
    for c in range(n_cores):
        b, half = c // halves, c % halves
        xTb = np.asarray(x[b], np.float32).T  # [D, S]
        if half:
            xTb = np.roll(xTb, -half * SQ, axis=1)
        hi, lo = _split8(xTb)
        hi = hi.reshape(ET, P, S)
        lo = lo.reshape(ET, P, S)
        x8 = np.empty((P, ET, 2, S), F8NP)
        x8[:, :, 0] = hi.transpose(1, 0, 2)
        x8[:, :, 1] = lo.transpose(1, 0, 2)
        in_maps.append({"x8": np.ascontiguousarray(x8), **common})
    return in_maps, SQ


def kernel(x, Wq, bq, Wk, bk, Wv, bv, Wo, bo, ln_w, ln_b, _trace=False):
    x = np.asarray(x)
    B, S, D = x.shape
    n_cores = 8
    in_maps, SQ = make_in_maps(x, Wq, bq, Wk, bk, Wv, bv, Wo, bo, ln_w, ln_b,
                               n_cores)
    nc = _get_nc(S, SQ, D, 16)
    res = run_bass_kernel_spmd(nc, in_maps, list(range(n_cores)), trace=_trace)
    out = np.empty((B, S, D), np.float32)
    halves = n_cores // B
    for c in range(n_cores):
        b, half = c // halves, c % halves
        out[b, half * SQ:(half + 1) * SQ] = res.results[c]["out"]
    out *= np.asarray(ln_w, np.float32)
    out += np.asarray(ln_b, np.float32)
    kernel.last_result = res
    return out


if __name__ == "__main__":
    nc = build_bass(2048, 1024, 1024, 16)
    print("built ok")


# revision 65
# speedup vs baseline: 1.0105x; 1.0105x over previous
"""Trainium2 Bass kernel for MultiHeadAttention (B=4, S=2048, D=1024, H=16).

Sharding: 8 cores = 4 batches x 2 sequence-halves, no collectives. Each
core computes full k/v projections for its batch and q/attention/
out-proj/LayerNorm for its half of the sequence. A host-side column
roll of x^T makes the program identical across cores (softmax over
keys is permutation-invariant): the core's q rows are always columns
[0, SQ) of its x^T.

Device program (per core):

Projections run as residual-fp8 DoubleRow matmuls: host splits x and
W (W pre-scaled by 32 so e4m3 normals cover it) into e4m3 hi+lo
planes; W.x ~= (W_hi+W_lo).x_hi + W_hi.x_lo (the dropped lo.lo term
is ~0.1%). Each K=128 bf16 matmul pair becomes a K=256 DoubleRow
fp8 matmul at half cost, 0.75x total. PSUM holds 32x the result; the
PSUM->SBUF bias-add copy folds in the 1/32 (v keeps the 32x scale,
cancelled by a 32-valued "ones" column in the softmax denominator).

Attention per head pair m (heads 2m/2m+1 on partition halves of
kt/qt), per q-chunk n:
  scT[j]  = kT_h[:, j-tile] . qT_h    both heads -> one 2-bank PSUM
  eT[j]   = exp(scT[j]/8)             one wide ScalarE op, bf16
  av[qs] += eT[j]_h[:, qs-tile].T @ v_aug[j]_h   [128q, 65] PSUM:
            e is the stationary operand so the moving free dim is 65
            (=FLOP-ideal; the old orientation re-streamed q per key
            tile at 2x cost). Row 64 = denominator via the 32-column.
  aotB    = av[:, 0:64] / av[:, 64]   per-partition scalar divide
  aotT    = PE-transpose(aotB)        -> aot [d, q] for the out-proj
out = aotT.T @ Wo.T + bo; LayerNorm (rstd via Ln+Exp so ScalarE stays
on one activation table set) -> DRAM.
"""

import os
import sys
from contextlib import ExitStack

for _p in ("/opt/trn_rl_repo", "/root/.axon_site/_ro/trn_rl_repo"):
    if _p not in sys.path and os.path.isdir(_p):
        sys.path.insert(0, _p)

# The kernel executes through the axon jax platform; a cpu-pinned
# JAX_PLATFORMS (used for running references) would hide the NeuronCores.
if "jax" not in sys.modules and "axon" not in os.environ.get(
        "JAX_PLATFORMS", "axon"):
    os.environ.pop("JAX_PLATFORMS")

import ml_dtypes
import numpy as np

import concourse.bacc as bacc
import concourse.mybir as mybir
import concourse.tile as tile
from concourse import library_config
from concourse.bass_utils import run_bass_kernel_spmd

BF16 = mybir.dt.bfloat16
F32 = mybir.dt.float32
F8 = mybir.dt.float8e4
AF = mybir.ActivationFunctionType
ALU = mybir.AluOpType

HD = 64      # head dim
WSCALE = 32.0  # host pre-scale on projection weights (fp8 range)


def build_bass(S, SQ, D, H, dtype=BF16, debug=False):
    """Build the per-core Bass program. S: kv seq len, SQ: q rows handled
    by this core, D: embed dim, H: total heads."""
    assert D == H * HD
    P = 128
    ET = D // P           # contraction tiles; also head-pair count
    QC = min(512, SQ)     # q free-dim chunk
    QN = SQ // QC
    KC = min(512, S)
    KN = S // KC
    VC = min(512, D)      # v/out-proj d chunk
    VN = D // VC
    HPC = VC // HD        # heads per v chunk
    SJ = S // P           # key tiles
    TQ = SQ // P          # q row tiles
    QS = QC // P          # q sub-tiles per chunk (AV stationary slices)

    nc = bacc.Bacc("TRN2", debug=False)

    x8 = nc.dram_tensor("x8", [P, ET, 2, S], F8, kind="ExternalInput").ap()
    ws = {}
    for w in ("wq8", "wk8"):   # per head-pair m: [m, p, kt*2*d]
        ws[w] = nc.dram_tensor(w, [ET, P, ET * 2 * P], F8,
                               kind="ExternalInput").ap()
    for w in ("wqh8", "wkh8"):  # hi-only copies for the cross term
        ws[w] = nc.dram_tensor(w, [ET, P, ET * P], F8,
                               kind="ExternalInput").ap()
    ws["wv8"] = nc.dram_tensor("wv8", [P, ET * 2 * D], F8,
                               kind="ExternalInput").ap()
    ws["wvh8"] = nc.dram_tensor("wvh8", [P, ET * D], F8,
                                kind="ExternalInput").ap()
    ws["wo"] = nc.dram_tensor("wo", [D, D], dtype, kind="ExternalInput").ap()
    ws["ident"] = nc.dram_tensor("ident", [P, P], dtype,
                                 kind="ExternalInput").ap()
    bs = {
        b: nc.dram_tensor(b, [D], F32, kind="ExternalInput").ap()
        for b in ("bq", "bk")
    }
    bs.update({
        b: nc.dram_tensor(b, [D], dtype, kind="ExternalInput").ap()
        for b in ("bv", "bo")
    })
    out = nc.dram_tensor("out", [SQ, D], F32, kind="ExternalOutput").ap()
    dbg = {}
    if debug:
        for nm, shp in (("dq0", [P, SQ]), ("dk0", [P, S]),
                        ("dv", [P, SJ * H * (HD + 1)]),
                        ("daot", [P, ET * SQ]),
                        ("det0", [P, 2 * QC]), ("dab0", [P, P])):
            dbg[nm] = nc.dram_tensor(nm, shp, BF16, kind="ExternalOutput").ap()

    with tile.TileContext(nc) as tc, ExitStack() as ctx:
        singles = ctx.enter_context(tc.tile_pool(name="singles", bufs=1))
        xp = ctx.enter_context(tc.tile_pool(name="xp", bufs=1))
        qtp = ctx.enter_context(tc.tile_pool(name="qtp", bufs=3))
        ktp = ctx.enter_context(tc.tile_pool(name="ktp", bufs=3))
        qkv = ctx.enter_context(tc.tile_pool(name="qkv", bufs=1))
        wqk = ctx.enter_context(tc.tile_pool(name="wqk", bufs=2))
        wvp = ctx.enter_context(tc.tile_pool(name="wvp", bufs=1))
        wop = ctx.enter_context(tc.tile_pool(name="wop", bufs=1))
        expp = ctx.enter_context(tc.tile_pool(name="expp", bufs=18))
        aotbp = ctx.enter_context(tc.tile_pool(name="aotbp", bufs=4))
        outp = ctx.enter_context(tc.tile_pool(name="outp", bufs=2))
        lnp = ctx.enter_context(tc.tile_pool(name="lnp", bufs=5))
        mmp = ctx.enter_context(tc.tile_pool(name="mm", bufs=2, space="PSUM"))
        scp = ctx.enter_context(tc.tile_pool(name="scp", bufs=2, space="PSUM"))
        avp = ctx.enter_context(tc.tile_pool(name="avp", bufs=1, space="PSUM"))

        nc.gpsimd.load_library(library_config.proxy)

        vt = qkv.tile([P, SJ, H, HD + 1], dtype, tag="vt")
        aot = qkv.tile([P, ET, SQ], dtype, tag="aot")
        qts, kts = {}, {}

        def load_wqk(m, eng=None):
            eng = eng or nc.gpsimd
            tiles = []
            for w, wh in (("wq8", "wqh8"), ("wk8", "wkh8")):
                wm = wqk.tile([P, ET, 2, P], F8, tag=w, name=w)
                eng.dma_start(
                    wm, ws[w][m].rearrange("p (t i d) -> p t i d", i=2, d=P))
                whm = wqk.tile([P, ET, P], F8, tag=wh, name=wh)
                eng.dma_start(
                    whm, ws[wh][m].rearrange("p (t d) -> p t d", d=P))
                tiles += [wm, whm]
            return tiles

        # --- x^T fp8 planes (hi, hi, lo), alternating HWDGE queues so the
        # projections aren't DMA-starved at startup
        pre0 = load_wqk(0, nc.scalar)
        pre1 = load_wqk(1, nc.scalar)
        x8t = xp.tile([P, ET, 2, S], F8, tag="x8t")
        QTR = S // 4
        qs_engs = (nc.sync, nc.gpsimd, nc.scalar)
        di = 0
        for c in range(4):
            for k in range(ET):
                eng = qs_engs[di % 3]
                di += 1
                eng.dma_start(
                    x8t[:, k, :, c * QTR:(c + 1) * QTR],
                    x8[:, k, :, c * QTR:(c + 1) * QTR])

        # --- constants ---
        bqk = singles.tile([P, 2 * ET], F32, tag="bqk")
        nc.sync.dma_start(bqk[:, :ET], bs["bq"].rearrange("(t p) -> p t", p=P))
        nc.sync.dma_start(bqk[:, ET:], bs["bk"].rearrange("(t p) -> p t", p=P))
        ident = singles.tile([P, P], dtype, tag="ident")
        nc.sync.dma_start(ident, ws["ident"])
        # free-dim bias rows, physically replicated across partitions
        brow = {}
        for b in ("bv", "bo"):
            t = singles.tile([P, D], dtype, tag=b)
            nc.sync.dma_start(t[0:1, :], bs[b][None, :])
            nc.gpsimd.partition_broadcast(t, t[0:1, :])
            brow[b] = t
        # denominator column: v carries a 32x scale, so the "ones" are 32
        nc.vector.memset(vt[:, :, :, HD:HD + 1], WSCALE)

        DR = mybir.MatmulPerfMode.DoubleRow

        urgent_q = []  # work needed promptly (v0 chunks, transposes):
                       # one item per attention slot, ahead of paced fill
        fill_q = []    # (cost_ns, closure) projection work, drained inside
                       # the attention j-loops at a credit-based rate sized
                       # to last through ALL the blocks
        slots_left = [0]
        fill_state = [0.0, 0.0]  # total queued cost, accumulated credit

        TOT_SLOTS = [0]

        def drain_fill():
            if urgent_q:
                urgent_q.pop(0)()
            if not fill_q:
                return
            cur = TOT_SLOTS[0] - slots_left[0]
            fill_state[1] += fill_state[0] / max(slots_left[0], 1)
            n = 0
            while fill_q and n < 3:
                c, fn, dl = fill_q[0]
                # pop on accumulated credit, or when the consumer block is
                # near (deadline keeps pool-rotation overwrites ordered
                # after the delayed chunk reads)
                if fill_state[1] >= c or dl - 16 <= cur:
                    fill_q.pop(0)
                    fill_state[0] -= c
                    fill_state[1] = max(0.0, fill_state[1] - c)
                    fn()
                    n += 1
                else:
                    break

        def queue_fill(cost, fn, deadline=10**9):
            fill_q.append((cost, fn, deadline))
            fill_state[0] += cost

        def qk_proj(m, pre=None, defer=False):
            dl = m * 2 * SJ
            """q and k projections for head-pair m (residual-fp8 DR)."""
            wqm, wqhm, wkm, wkhm = pre if pre is not None else load_wqk(m)
            qt = qtp.tile([P, SQ], dtype, tag="qt", name="qt")
            kt = ktp.tile([P, S], dtype, tag="kt", name="kt")
            qts[m], kts[m] = qt, kt
            order = [(0, 0), (1, 0), (1, 1), (0, 1), (1, 2), (1, 3)]
            specs = ((qt, wqm, wqhm, QN, QC, m),
                     (kt, wkm, wkhm, KN, KC, ET + m))
            for which, n in order:
                dst, wm, whm, nch, cc, bofs = specs[which]
                if True:
                    def chunk(dst=dst, wm=wm, whm=whm, n=n, cc=cc, bofs=bofs):
                        cols = slice(n * cc, (n + 1) * cc)
                        ps = mmp.tile([P, 512], F32, tag="mm", name="ps")[:, :cc]
                        for k in range(ET):
                            nc.tensor.matmul(
                                ps, wm[:, k], x8t[:, k, 0:2, cols],
                                start=(k == 0), stop=False, perf_mode=DR)
                        for kp in range(ET // 2):
                            nc.tensor.matmul(
                                ps, whm[:, 2 * kp:2 * kp + 2, :],
                                x8t[:, 2 * kp:2 * kp + 2, 0, cols],
                                start=False, stop=(kp == ET // 2 - 1),
                                perf_mode=DR)
                        nc.vector.tensor_scalar(
                            dst[:, cols], ps, 1.0 / WSCALE,
                            bqk[:, bofs:bofs + 1], ALU.mult, ALU.add)
                    if defer:
                        queue_fill(1300, chunk, dl)
                    else:
                        chunk()

        def load_wv(n, eng=None):
            eng = eng or nc.gpsimd
            wvn = wvp.tile([P, ET, 2, VC], F8, tag="wv", name="wvn")
            eng.dma_start(
                wvn,
                ws["wv8"].rearrange("p (t i d) -> p t i d", i=2, d=D)
                [:, :, :, n * VC:(n + 1) * VC])
            wvhn = wvp.tile([P, ET, VC], F8, tag="wvh", name="wvhn")
            eng.dma_start(
                wvhn,
                ws["wvh8"].rearrange("p (t d) -> p t d", d=D)
                [:, :, n * VC:(n + 1) * VC])
            return wvn, wvhn

        def v_block(n, wvn, wvhn, j):
            """v projection d-chunk n, s-tile j (keeps 32x scale)."""
            jcols = slice(j * P, (j + 1) * P)
            ps = mmp.tile([P, 512], F32, tag="mm", name="ps")[:, :VC]
            for k in range(ET):
                nc.tensor.matmul(
                    ps, x8t[:, k, 0:2, jcols], wvn[:, k],
                    start=(k == 0), stop=False, perf_mode=DR)
            for kp in range(ET // 2):
                nc.tensor.matmul(
                    ps, x8t[:, 2 * kp:2 * kp + 2, 0, jcols],
                    wvhn[:, 2 * kp:2 * kp + 2, :],
                    start=False, stop=(kp == ET // 2 - 1), perf_mode=DR)
            nc.vector.tensor_tensor(
                vt[:, j, n * HPC:(n + 1) * HPC, 0:HD],
                ps.rearrange("p (h d) -> p h d", d=HD),
                brow["bv"][:, n * VC:(n + 1) * VC].rearrange(
                    "p (h d) -> p h d", d=HD),
                ALU.add,
            )

        def v_proj(n, wv, defer=False):
            for j in range(SJ):
                if defer == "urgent":
                    urgent_q.append(
                        lambda n=n, wv=wv, j=j: v_block(n, wv[0], wv[1], j))
                elif defer:
                    queue_fill(
                        1300,
                        lambda n=n, wv=wv, j=j: v_block(n, wv[0], wv[1], j),
                        (H // HPC // 2) * 2 * SJ)
                else:
                    v_block(n, wv[0], wv[1], j)

        def att_exp(m, n, j):
            """score pair + exp for (head pair m, q-chunk n, k-tile j)."""
            qt, kt = qts[m], kts[m]
            sc = scp.tile([P, 2, 512], F32, tag="sc", name="sc")
            nc.tensor.matmul(
                sc[:, 0, :QC],
                kt[0:HD, j * P:(j + 1) * P],
                qt[0:HD, n * QC:(n + 1) * QC],
            )
            nc.tensor.matmul(
                sc[:, 1, :QC],
                kt[HD:P, j * P:(j + 1) * P],
                qt[HD:P, n * QC:(n + 1) * QC],
            )
            et = expp.tile([P, 2, QC], dtype, tag="exp", name="et")
            nc.scalar.activation(et, sc[:, :, :QC], AF.Exp, scale=0.125)
            if debug and m == 0 and n == 0 and j == 0:
                nc.sync.dma_start(dbg["det0"], et.rearrange("p a b -> p (a b)"))
            return et

        def av_mms(m, et, j, av4):
            # av4 slices padded to 512B so no matmul output crosses a
            # 2KB PSUM bank boundary. start=True clears the WHOLE bank,
            # so only the first matmul touching each bank carries it; the
            # other slices' first write lands on cleared has_written bits
            # and overwrites.
            for qs in range(QS):
                for h in range(2):
                    k = 2 * qs + h
                    nc.tensor.matmul(
                        av4[:, k, 0:HD + 1],
                        et[:, h, qs * P:(qs + 1) * P],
                        vt[:, j, 2 * m + h, :],
                        start=(j == 0 and k % 4 == 0), stop=(j == SJ - 1),
                        skip_group_check=True,
                    )

        tail_work = []  # previous block's last AV group + normalize

        def block_tail(m, n, pend, av4):
            """Final AV matmuls + per-head normalize of a block. Deferred
            until after the NEXT block's first scores so the PE never
            stalls on the block's last exps."""
            for j, et in pend:
                av_mms(m, et, j, av4)
            for qs in range(QS):
                t = n * QS + qs
                ab = aotbp.tile([P, 2 * HD], dtype, tag="ab", name="ab")
                rcp = aotbp.tile([P, 2], F32, tag="rcp", name="rcp")
                for h in range(2):
                    k = 2 * qs + h
                    nc.vector.reciprocal(
                        rcp[:, h:h + 1], av4[:, k, HD:HD + 1])
                    nc.vector.tensor_scalar(
                        ab[:, h * HD:(h + 1) * HD], av4[:, k, 0:HD],
                        rcp[:, h:h + 1], None, ALU.mult)

                if debug and m == 0 and n == 0 and qs == 0:
                    nc.sync.dma_start(dbg["dab0"], ab)

                def finish(m=m, t=t, ab=ab):
                    tr = mmp.tile([P, P], dtype, tag="mm", name="tr")
                    nc.tensor.transpose(tr, ab, ident)
                    nc.vector.tensor_copy(aot[:, m, t * P:(t + 1) * P], tr)
                # run in upcoming slots so the PE transpose never queues
                # ahead of scores while its input is still pending
                urgent_q.append(finish)

        def attention(m, n):
            """q-chunk n of head pair m (heads 2m, 2m+1). The AV matmuls
            trail one key-tile behind their exp so they never head-of-line
            block PE's in-order SEQ; one projection chunk is drained from
            fill_q per key-tile to keep PE busy under the ScalarE exps."""
            av4 = avp.tile([P, 2 * QS, P, ], F32, tag="av", name="av")
            pend = []
            for j in range(SJ):
                et = att_exp(m, n, j)
                if j == 14 and tail_work:
                    tail_work.pop(0)()
                else:
                    drain_fill()
                slots_left[0] -= 1
                # AV trails its exp by two key-tiles so exp jitter never
                # head-of-line blocks PE's in-order SEQ
                if len(pend) >= 14:
                    pj, pet = pend.pop(0)
                    av_mms(m, pet, pj, av4)
                pend.append((j, et))
            tail_work.append(lambda: block_tail(m, n, pend, av4))

        ln_pend = {}

        def out_ln(t, part="all"):
            """Out-projection + LayerNorm for q row tile t. part="front"
            stops before the ScalarE rstd (so overlapping out_lns don't
            interleave table sets with the attention exps); "back" finishes."""
            if part == "back":
                ot, scr, mv, rstd = ln_pend.pop(t)
                nc.scalar.activation(rstd, mv[:, 1:2], AF.Abs_reciprocal_sqrt,
                                     bias=eps)
                for cc in range(VN):
                    cs = slice(cc * VC, (cc + 1) * VC)
                    nc.vector.tensor_scalar(
                        ot[:, cs], ot[:, cs], mv[:, 0:1], rstd,
                        ALU.subtract, ALU.mult)
                    nc.sync.dma_start(
                        out.rearrange("(t p) d -> p t d", p=P)[:, t, cs],
                        ot[:, cs])
                return
            FSUB = min(512, D)
            NSUB = D // FSUB
            ot = outp.tile([P, D], F32, tag="ot", name="ot")
            for nn in range(VN):
                # post-attention out-projs borrow the idle scores pool so
                # the tail double-buffers without contending with the
                # transpose finishers in mmp
                if t >= QS:
                    ps = scp.tile([P, 2, 512], F32, tag="sc",
                                  name="ps")[:, 0, :VC]
                else:
                    ps = mmp.tile([P, 512], F32, tag="mm", name="ps")[:, :VC]
                for k in range(ET):
                    nc.tensor.matmul(
                        ps, aot[:, k, t * P:(t + 1) * P],
                        wo[:, k, nn * VC:(nn + 1) * VC],
                        start=(k == 0), stop=(k == ET - 1),
                    )
                nc.vector.tensor_tensor(
                    ot[:, nn * VC:(nn + 1) * VC], ps,
                    brow["bo"][:, nn * VC:(nn + 1) * VC], ALU.add)
            scr = lnp.tile([P, NSUB * 6 + 3], F32, tag="scr", name="scr")
            stats = scr[:, 0:NSUB * 6].rearrange("p (s f) -> p s f", f=6)
            mv = scr[:, NSUB * 6:NSUB * 6 + 2]
            rstd = scr[:, NSUB * 6 + 2:NSUB * 6 + 3]
            otv = ot.rearrange("p (s f) -> p s f", f=FSUB)
            for sbi in range(NSUB):
                nc.vector.bn_stats(stats[:, sbi, :], otv[:, sbi, :])
            nc.vector.bn_aggr(mv, stats)
            if part == "front":
                ln_pend[t] = (ot, scr, mv, rstd)
                return
            nc.scalar.activation(rstd, mv[:, 1:2], AF.Abs_reciprocal_sqrt,
                                 bias=eps)
            # device emits the normalized rows; the ln_w/ln_b affine is a
            # pure elementwise epilogue applied host-side on the gathered
            # output. Chunked so each output DMA starts as soon as its
            # half is normalized.
            for cc in range(VN):
                cs = slice(cc * VC, (cc + 1) * VC)
                nc.vector.tensor_scalar(
                    ot[:, cs], ot[:, cs], mv[:, 0:1], rstd,
                    ALU.subtract, ALU.mult)
                nc.sync.dma_start(
                    out.rearrange("(t p) d -> p t d", p=P)[:, t, cs],
                    ot[:, cs])

        # --- emission schedule ---
        wo = wop.tile([P, ET, D], dtype, tag="wo")
        slots_left[0] = ET * QN * SJ
        TOT_SLOTS[0] = slots_left[0]
        qk_proj(0, pre0)
        wv0 = load_wv(0, nc.sync)
        v_proj(0, wv0, defer="urgent")
        qk_proj(1, pre1, defer=True)
        nc.gpsimd.dma_start(wo, ws["wo"].rearrange("(t p) d -> p t d", p=P))
        done_ln = [0]

        def queue_ln(upto, part="all"):
            while done_ln[0] < upto:
                t = done_ln[0]
                urgent_q.append(lambda t=t, part=part: out_ln(t, part))
                done_ln[0] += 1

        for m in range(ET):
            for n in range(QN):
                if m == ET - 1 and n == 1:
                    # first-half out-projs are ready: overlap them with
                    # the final attention block (ScalarE-free front part)
                    queue_ln(QS, "front")
                attention(m, n)
            # queue the remaining projections; the attention j-loops drain
            # them at a rate sized to last through all the blocks
            if m + 2 < ET:
                qk_proj(m + 2, defer=True)
            if m == 0:
                v_proj(1, load_wv(1), defer=True)
        if debug:
            nc.sync.dma_start(dbg["dq0"], qts[0])
            nc.sync.dma_start(dbg["dk0"], kts[0])
            nc.sync.dma_start(
                dbg["dv"], vt.rearrange("p a b c -> p (a b c)"))
        while tail_work:
            tail_work.pop(0)()
        # drain remaining finishers interleaved with the out_lns they gate
        while urgent_q:
            urgent_q.pop(0)()
            queue_ln(min(done_ln[0] + 1, TQ))
        for t in sorted(ln_pend):
            out_ln(t, "back")
        while fill_q:
            fill_q.pop(0)[1]()

        if debug:
            nc.sync.dma_start(dbg["daot"], aot.rearrange("p a b -> p (a b)"))
        # out-proj/LN remainder
        queue_ln(TQ)
        while urgent_q:
            urgent_q.pop(0)()

    nc.compile()
    return nc


# ---------------------------------------------------------------- host side

_CACHE = {}


def _get_nc(S, SQ, D, H):
    key = (S, SQ, D, H)
    if key not in _CACHE:
        _CACHE[key] = build_bass(S, SQ, D, H)
    return _CACHE[key]


F8NP = ml_dtypes.float8_e4m3


def _split8(a):
    """fp32 array -> (hi, lo) e4m3 planes with hi+lo ~= a."""
    hi = np.asarray(a, np.float32).astype(F8NP)
    lo = (np.asarray(a, np.float32) - hi.astype(np.float32)).astype(F8NP)
    return hi, lo


def make_in_maps(x, Wq, bq, Wk, bk, Wv, bv, Wo, bo, ln_w, ln_b, n_cores=8):
    """Shard full inputs into per-core input maps (batch x seq-half)."""
    B, S, D = x.shape
    halves = n_cores // B
    SQ = S // halves
    bf = ml_dtypes.bfloat16
    ET = D // 128
    P = 128

    def pack_qk(W):
        # base[m, p, t, d] = (W*32)[m*128+d, t*128+p]
        hi, lo = _split8(np.asarray(W, np.float32) * WSCALE)

        def arr(z):
            w4 = z.astype(np.float32).T.reshape(ET, P, ET, P)  # [t, p, m, d]
            return np.ascontiguousarray(w4.transpose(2, 1, 0, 3))  # [m,p,t,d]

        ah, al = arr(hi), arr(lo)
        full = np.stack([ah, ah], axis=3)  # [m, p, t, 2, d] (hi dup)
        return (full.reshape(ET, P, ET * 2 * P).astype(F8NP),
                np.ascontiguousarray(al).reshape(ET, P, ET * P).astype(F8NP))

    def pack_v(W):
        hi, lo = _split8(np.asarray(W, np.float32) * WSCALE)

        def arr(z):  # [p, t, dcol] = z.T[t*128+p, dcol]
            return np.ascontiguousarray(
                z.astype(np.float32).T.reshape(ET, P, D).transpose(1, 0, 2))

        ah, al = arr(hi), arr(lo)
        full = np.stack([ah, ah], axis=2)  # [p, t, 2, d] (hi dup)
        return (full.reshape(P, ET * 2 * D).astype(F8NP),
                al.reshape(P, ET * D).astype(F8NP))

    wq8, wqh8 = pack_qk(Wq)
    wk8, wkh8 = pack_qk(Wk)
    wv8, wvh8 = pack_v(Wv)
    common = {
        "wq8": wq8, "wqh8": wqh8, "wk8": wk8, "wkh8": wkh8,
        "wv8": wv8, "wvh8": wvh8,
        "wo": np.ascontiguousarray(np.asarray(Wo).T).astype(bf),
        "ident": np.eye(P, dtype=np.float32).astype(bf),
        "bq": np.asarray(bq, np.float32), "bk": np.asarray(bk, np.float32),
        "bv": (np.asarray(bv, np.float32) * WSCALE).astype(bf),
        "bo": np.asarray(bo, np.float32).astype(bf),
    }
    in_maps = []
    for c in range(n_cores):
        b, half = c // halves, c % halves
        xTb = np.asarray(x[b], np.float32).T  # [D, S]
        if half:
            xTb = np.roll(xTb, -half * SQ, axis=1)
        hi, lo = _split8(xTb)
        hi = hi.reshape(ET, P, S)
        lo = lo.reshape(ET, P, S)
        x8 = np.empty((P, ET, 2, S), F8NP)
        x8[:, :, 0] = hi.transpose(1, 0, 2)
        x8[:, :, 1] = lo.transpose(1, 0, 2)
        in_maps.append({"x8": np.ascontiguousarray(x8), **common})
    return in_maps, SQ


def kernel(x, Wq, bq, Wk, bk, Wv, bv, Wo, bo, ln_w, ln_b, _trace=False):
    x = np.asarray(x)
    B, S, D = x.shape
    n_cores = 8
    in_maps, SQ = make_in_maps(x, Wq, bq, Wk, bk, Wv, bv, Wo, bo, ln_w, ln_b,
                               n_cores)
    nc = _get_nc(S, SQ, D, 16)
    res = run_bass_kernel_spmd(nc, in_maps, list(range(n_cores)), trace=_trace)
    out = np.empty((B, S, D), np.float32)
    halves = n_cores // B
    for c in range(n_cores):
        b, half = c // halves, c % halves
        out[b, half * SQ:(half + 1) * SQ] = res.results[c]["out"]
    out *= np.asarray(ln_w, np.float32)
    out += np.asarray(ln_b, np.float32)
    kernel.last_result = res
    return out


if __name__ == "__main__":
    nc = build_bass(2048, 1024, 1024, 16)
    print("built ok")


# revision 81
# speedup vs baseline: 1.0184x; 1.0078x over previous
"""Trainium2 Bass kernel for MultiHeadAttention (B=4, S=2048, D=1024, H=16).

Sharding: 8 cores = 4 batches x 2 sequence-halves, no collectives. Each
core computes full k/v projections for its batch and q/attention/
out-proj/LayerNorm for its half of the sequence. A host-side column
roll of x^T makes the program identical across cores (softmax over
keys is permutation-invariant): the core's q rows are always columns
[0, SQ) of its x^T.

Device program (per core):

Projections run as residual-fp8 DoubleRow matmuls: host splits x and
W (W pre-scaled by 32 so e4m3 normals cover it) into e4m3 hi+lo
planes; W.x ~= (W_hi+W_lo).x_hi + W_hi.x_lo (the dropped lo.lo term
is ~0.1%). Each K=128 bf16 matmul pair becomes a K=256 DoubleRow
fp8 matmul at half cost, 0.75x total. PSUM holds 32x the result; the
PSUM->SBUF bias-add copy folds in the 1/32 (v keeps the 32x scale,
cancelled by a 32-valued "ones" column in the softmax denominator).

Attention per head pair m (heads 2m/2m+1 on partition halves of
kt/qt), per q-chunk n:
  scT[j]  = kT_h[:, j-tile] . qT_h    both heads -> one 2-bank PSUM
  eT[j]   = exp(scT[j]/8)             one wide ScalarE op, bf16
  av[qs] += eT[j]_h[:, qs-tile].T @ v_aug[j]_h   [128q, 65] PSUM:
            e is the stationary operand so the moving free dim is 65
            (=FLOP-ideal; the old orientation re-streamed q per key
            tile at 2x cost). Row 64 = denominator via the 32-column.
  aotB    = av[:, 0:64] / av[:, 64]   per-partition scalar divide
  aotT    = PE-transpose(aotB)        -> aot [d, q] for the out-proj
out = aotT.T @ Wo.T + bo; LayerNorm (rstd via Ln+Exp so ScalarE stays
on one activation table set) -> DRAM.
"""

import os
import sys
from contextlib import ExitStack

for _p in ("/opt/trn_rl_repo", "/root/.axon_site/_ro/trn_rl_repo"):
    if _p not in sys.path and os.path.isdir(_p):
        sys.path.insert(0, _p)

# The kernel executes through the axon jax platform; a cpu-pinned
# JAX_PLATFORMS (used for running references) would hide the NeuronCores.
if "jax" not in sys.modules and "axon" not in os.environ.get(
        "JAX_PLATFORMS", "axon"):
    os.environ.pop("JAX_PLATFORMS")

import ml_dtypes
import numpy as np

import concourse.bacc as bacc
import concourse.mybir as mybir
import concourse.tile as tile
from concourse import library_config
from concourse.bass_utils import run_bass_kernel_spmd

BF16 = mybir.dt.bfloat16
F32 = mybir.dt.float32
F8 = mybir.dt.float8e4
AF = mybir.ActivationFunctionType
ALU = mybir.AluOpType

HD = 64      # head dim
WSCALE = 32.0  # host pre-scale on projection weights (fp8 range)


def build_bass(S, SQ, D, H, dtype=BF16, debug=False):
    """Build the per-core Bass program. S: kv seq len, SQ: q rows handled
    by this core, D: embed dim, H: total heads."""
    assert D == H * HD
    P = 128
    ET = D // P           # contraction tiles; also head-pair count
    QC = min(512, SQ)     # q free-dim chunk
    QN = SQ // QC
    KC = min(512, S)
    KN = S // KC
    VC = min(512, D)      # v/out-proj d chunk
    VN = D // VC
    HPC = VC // HD        # heads per v chunk
    SJ = S // P           # key tiles
    TQ = SQ // P          # q row tiles
    QS = QC // P          # q sub-tiles per chunk (AV stationary slices)

    nc = bacc.Bacc("TRN2", debug=False)

    x8 = nc.dram_tensor("x8", [P, ET, 2, S], F8, kind="ExternalInput").ap()
    ws = {}
    for w in ("wq8", "wk8"):   # per head-pair m: [m, p, kt*2*d]
        ws[w] = nc.dram_tensor(w, [ET, P, ET * 2 * P], F8,
                               kind="ExternalInput").ap()
    for w in ("wqh8", "wkh8"):  # hi-only copies for the cross term
        ws[w] = nc.dram_tensor(w, [ET, P, ET * P], F8,
                               kind="ExternalInput").ap()
    ws["wv8"] = nc.dram_tensor("wv8", [P, ET * 2 * D], F8,
                               kind="ExternalInput").ap()
    ws["wvh8"] = nc.dram_tensor("wvh8", [P, ET * D], F8,
                                kind="ExternalInput").ap()
    ws["wo"] = nc.dram_tensor("wo", [D, D], dtype, kind="ExternalInput").ap()
    ws["ident"] = nc.dram_tensor("ident", [P, P], dtype,
                                 kind="ExternalInput").ap()
    bs = {
        b: nc.dram_tensor(b, [D], F32, kind="ExternalInput").ap()
        for b in ("bq", "bk")
    }
    bs.update({
        b: nc.dram_tensor(b, [D], dtype, kind="ExternalInput").ap()
        for b in ("bv", "bo")
    })
    out = nc.dram_tensor("out", [SQ, D], F32, kind="ExternalOutput").ap()
    dbg = {}
    if debug:
        for nm, shp in (("dq0", [P, SQ]), ("dk0", [P, S]),
                        ("dv", [P, SJ * H * (HD + 1)]),
                        ("daot", [P, ET * SQ]),
                        ("det0", [P, 2 * QC]), ("dab0", [P, P])):
            dbg[nm] = nc.dram_tensor(nm, shp, BF16, kind="ExternalOutput").ap()

    with tile.TileContext(nc) as tc, ExitStack() as ctx:
        singles = ctx.enter_context(tc.tile_pool(name="singles", bufs=1))
        xp = ctx.enter_context(tc.tile_pool(name="xp", bufs=1))
        qtp = ctx.enter_context(tc.tile_pool(name="qtp", bufs=3))
        ktp = ctx.enter_context(tc.tile_pool(name="ktp", bufs=3))
        qkv = ctx.enter_context(tc.tile_pool(name="qkv", bufs=1))
        wqk = ctx.enter_context(tc.tile_pool(name="wqk", bufs=2))
        wvp = ctx.enter_context(tc.tile_pool(name="wvp", bufs=1))
        wop = ctx.enter_context(tc.tile_pool(name="wop", bufs=1))
        expp = ctx.enter_context(tc.tile_pool(name="expp", bufs=20))
        aotbp = ctx.enter_context(tc.tile_pool(name="aotbp", bufs=4))
        outp = ctx.enter_context(tc.tile_pool(name="outp", bufs=2))
        lnp = ctx.enter_context(tc.tile_pool(name="lnp", bufs=8))
        mmp = ctx.enter_context(tc.tile_pool(name="mm", bufs=2, space="PSUM"))
        scp = ctx.enter_context(tc.tile_pool(name="scp", bufs=2, space="PSUM"))
        avp = ctx.enter_context(tc.tile_pool(name="avp", bufs=1, space="PSUM"))

        nc.gpsimd.load_library(library_config.proxy)

        vt = qkv.tile([P, SJ, H, HD + 1], dtype, tag="vt")
        aot = qkv.tile([P, ET, SQ], dtype, tag="aot")
        qts, kts = {}, {}

        def load_wqk(m, eng=None):
            eng = eng or nc.gpsimd
            tiles = []
            for w, wh in (("wq8", "wqh8"), ("wk8", "wkh8")):
                wm = wqk.tile([P, ET, 2, P], F8, tag=w, name=w)
                eng.dma_start(
                    wm, ws[w][m].rearrange("p (t i d) -> p t i d", i=2, d=P))
                whm = wqk.tile([P, ET, P], F8, tag=wh, name=wh)
                eng.dma_start(
                    whm, ws[wh][m].rearrange("p (t d) -> p t d", d=P))
                tiles += [wm, whm]
            return tiles

        # --- x^T fp8 planes (hi, hi, lo), alternating HWDGE queues so the
        # projections aren't DMA-starved at startup
        pre0 = load_wqk(0, nc.scalar)
        pre1 = load_wqk(1, nc.scalar)
        x8t = xp.tile([P, ET, 2, S], F8, tag="x8t")
        QTR = S // 4
        qs_engs = (nc.sync, nc.gpsimd, nc.scalar)
        di = 0
        for c in range(4):
            for k in range(ET):
                eng = qs_engs[di % 3]
                di += 1
                eng.dma_start(
                    x8t[:, k, :, c * QTR:(c + 1) * QTR],
                    x8[:, k, :, c * QTR:(c + 1) * QTR])

        # --- constants ---
        bqk = singles.tile([P, 2 * ET], F32, tag="bqk")
        nc.sync.dma_start(bqk[:, :ET], bs["bq"].rearrange("(t p) -> p t", p=P))
        nc.sync.dma_start(bqk[:, ET:], bs["bk"].rearrange("(t p) -> p t", p=P))
        ident = singles.tile([P, P], dtype, tag="ident")
        nc.sync.dma_start(ident, ws["ident"])
        # free-dim bias rows, physically replicated across partitions
        brow = {}
        for b in ("bv", "bo"):
            t = singles.tile([P, D], dtype, tag=b)
            nc.sync.dma_start(t[0:1, :], bs[b][None, :])
            nc.gpsimd.partition_broadcast(t, t[0:1, :])
            brow[b] = t
        # denominator column: v carries a 32x scale, so the "ones" are 32
        nc.vector.memset(vt[:, :, :, HD:HD + 1], WSCALE)

        DR = mybir.MatmulPerfMode.DoubleRow

        urgent_q = []  # work needed promptly (v0 chunks, transposes):
                       # one item per attention slot, ahead of paced fill
        fill_q = []    # (cost_ns, closure) projection work, drained inside
                       # the attention j-loops at a credit-based rate sized
                       # to last through ALL the blocks
        slots_left = [0]
        fill_state = [0.0, 0.0]  # total queued cost, accumulated credit

        TOT_SLOTS = [0]

        def drain_fill():
            if urgent_q:
                urgent_q.pop(0)()
            if not fill_q:
                return
            cur = TOT_SLOTS[0] - slots_left[0]
            fill_state[1] += fill_state[0] / max(slots_left[0], 1)
            n = 0
            while fill_q and n < 3:
                c, fn, dl = fill_q[0]
                # pop on accumulated credit, or when the consumer block is
                # near (deadline keeps pool-rotation overwrites ordered
                # after the delayed chunk reads)
                if fill_state[1] >= c or dl - 16 <= cur:
                    fill_q.pop(0)
                    fill_state[0] -= c
                    fill_state[1] = max(0.0, fill_state[1] - c)
                    fn()
                    n += 1
                else:
                    break

        def queue_fill(cost, fn, deadline=10**9):
            fill_q.append((cost, fn, deadline))
            fill_state[0] += cost

        def qk_proj(m, pre=None, defer=False):
            dl = m * 2 * SJ
            """q and k projections for head-pair m (residual-fp8 DR)."""
            wqm, wqhm, wkm, wkhm = pre if pre is not None else load_wqk(m)
            qt = qtp.tile([P, SQ], dtype, tag="qt", name="qt")
            kt = ktp.tile([P, S], dtype, tag="kt", name="kt")
            qts[m], kts[m] = qt, kt
            order = [(0, 0), (1, 0), (1, 1), (0, 1), (1, 2), (1, 3)]
            specs = ((qt, wqm, wqhm, QN, QC, m),
                     (kt, wkm, wkhm, KN, KC, ET + m))
            for which, n in order:
                dst, wm, whm, nch, cc, bofs = specs[which]
                if True:
                    def chunk(dst=dst, wm=wm, whm=whm, n=n, cc=cc, bofs=bofs):
                        cols = slice(n * cc, (n + 1) * cc)
                        ps = mmp.tile([P, 512], F32, tag="mm", name="ps")[:, :cc]
                        for k in range(ET):
                            nc.tensor.matmul(
                                ps, wm[:, k], x8t[:, k, 0:2, cols],
                                start=(k == 0), stop=False, perf_mode=DR)
                        for kp in range(ET // 2):
                            nc.tensor.matmul(
                                ps, whm[:, 2 * kp:2 * kp + 2, :],
                                x8t[:, 2 * kp:2 * kp + 2, 0, cols],
                                start=False, stop=(kp == ET // 2 - 1),
                                perf_mode=DR)
                        nc.vector.tensor_scalar(
                            dst[:, cols], ps, 1.0 / WSCALE,
                            bqk[:, bofs:bofs + 1], ALU.mult, ALU.add)
                    if defer:
                        queue_fill(1300, chunk, dl)
                    else:
                        chunk()

        def load_wv(n, eng=None):
            eng = eng or nc.gpsimd
            wvn = wvp.tile([P, ET, 2, VC], F8, tag="wv", name="wvn")
            eng.dma_start(
                wvn,
                ws["wv8"].rearrange("p (t i d) -> p t i d", i=2, d=D)
                [:, :, :, n * VC:(n + 1) * VC])
            wvhn = wvp.tile([P, ET, VC], F8, tag="wvh", name="wvhn")
            eng.dma_start(
                wvhn,
                ws["wvh8"].rearrange("p (t d) -> p t d", d=D)
                [:, :, n * VC:(n + 1) * VC])
            return wvn, wvhn

        def v_block(n, wvn, wvhn, j):
            """v projection d-chunk n, s-tile j (keeps 32x scale)."""
            jcols = slice(j * P, (j + 1) * P)
            ps = mmp.tile([P, 512], F32, tag="mm", name="ps")[:, :VC]
            for k in range(ET):
                nc.tensor.matmul(
                    ps, x8t[:, k, 0:2, jcols], wvn[:, k],
                    start=(k == 0), stop=False, perf_mode=DR)
            for kp in range(ET // 2):
                nc.tensor.matmul(
                    ps, x8t[:, 2 * kp:2 * kp + 2, 0, jcols],
                    wvhn[:, 2 * kp:2 * kp + 2, :],
                    start=False, stop=(kp == ET // 2 - 1), perf_mode=DR)
            nc.vector.tensor_tensor(
                vt[:, j, n * HPC:(n + 1) * HPC, 0:HD],
                ps.rearrange("p (h d) -> p h d", d=HD),
                brow["bv"][:, n * VC:(n + 1) * VC].rearrange(
                    "p (h d) -> p h d", d=HD),
                ALU.add,
            )

        def v_proj(n, wv, defer=False):
            for j in range(SJ):
                if defer == "urgent":
                    urgent_q.append(
                        lambda n=n, wv=wv, j=j: v_block(n, wv[0], wv[1], j))
                elif defer:
                    queue_fill(
                        1300,
                        lambda n=n, wv=wv, j=j: v_block(n, wv[0], wv[1], j),
                        (H // HPC // 2) * 2 * SJ)
                else:
                    v_block(n, wv[0], wv[1], j)

        def att_exp(m, n, j):
            """score pair + exp for (head pair m, q-chunk n, k-tile j)."""
            qt, kt = qts[m], kts[m]
            sc = scp.tile([P, 2, 512], F32, tag="sc", name="sc")
            nc.tensor.matmul(
                sc[:, 0, :QC],
                kt[0:HD, j * P:(j + 1) * P],
                qt[0:HD, n * QC:(n + 1) * QC],
            )
            nc.tensor.matmul(
                sc[:, 1, :QC],
                kt[HD:P, j * P:(j + 1) * P],
                qt[HD:P, n * QC:(n + 1) * QC],
            )
            et = expp.tile([P, 2, QC], dtype, tag="exp", name="et")
            nc.scalar.activation(et, sc[:, :, :QC], AF.Exp, scale=0.125)
            if debug and m == 0 and n == 0 and j == 0:
                nc.sync.dma_start(dbg["det0"], et.rearrange("p a b -> p (a b)"))
            return et

        def av_mms(m, et, j, av4):
            # av4 slices padded to 512B so no matmul output crosses a
            # 2KB PSUM bank boundary. start=True clears the WHOLE bank,
            # so only the first matmul touching each bank carries it; the
            # other slices' first write lands on cleared has_written bits
            # and overwrites.
            for qs in range(QS):
                for h in range(2):
                    k = 2 * qs + h
                    nc.tensor.matmul(
                        av4[:, k, 0:HD + 1],
                        et[:, h, qs * P:(qs + 1) * P],
                        vt[:, j, 2 * m + h, :],
                        start=(j == 0 and k % 4 == 0), stop=(j == SJ - 1),
                        skip_group_check=True,
                    )

        tail_work = []  # previous block's last AV group + normalize

        def block_tail(m, n, pend, av4):
            """Final AV matmuls + per-head normalize of a block. Deferred
            until after the NEXT block's first scores so the PE never
            stalls on the block's last exps."""
            for j, et in pend:
                av_mms(m, et, j, av4)
            for qs in range(QS):
                t = n * QS + qs
                ab = aotbp.tile([P, 2 * HD], dtype, tag="ab", name="ab")
                rcp = aotbp.tile([P, 2], F32, tag="rcp", name="rcp")
                for h in range(2):
                    k = 2 * qs + h
                    nc.vector.reciprocal(
                        rcp[:, h:h + 1], av4[:, k, HD:HD + 1])
                    nc.vector.tensor_scalar(
                        ab[:, h * HD:(h + 1) * HD], av4[:, k, 0:HD],
                        rcp[:, h:h + 1], None, ALU.mult)

                if debug and m == 0 and n == 0 and qs == 0:
                    nc.sync.dma_start(dbg["dab0"], ab)

                def finish(m=m, t=t, ab=ab):
                    tr = mmp.tile([P, P], dtype, tag="mm", name="tr")
                    nc.tensor.transpose(tr, ab, ident)
                    nc.vector.tensor_copy(aot[:, m, t * P:(t + 1) * P], tr)
                # run in upcoming slots so the PE transpose never queues
                # ahead of scores while its input is still pending
                urgent_q.append(finish)

        def attention(m, n):
            """q-chunk n of head pair m (heads 2m, 2m+1). The AV matmuls
            trail one key-tile behind their exp so they never head-of-line
            block PE's in-order SEQ; one projection chunk is drained from
            fill_q per key-tile to keep PE busy under the ScalarE exps."""
            av4 = avp.tile([P, 2 * QS, P, ], F32, tag="av", name="av")
            pend = []
            for j in range(SJ):
                et = att_exp(m, n, j)
                if j == 15 and tail_work:
                    tail_work.pop(0)()
                else:
                    drain_fill()
                slots_left[0] -= 1
                # AV trails its exp by two key-tiles so exp jitter never
                # head-of-line blocks PE's in-order SEQ
                if len(pend) >= 15:
                    pj, pet = pend.pop(0)
                    av_mms(m, pet, pj, av4)
                pend.append((j, et))
            tail_work.append(lambda: block_tail(m, n, pend, av4))

        ln_pend = {}

        def out_ln(t, part="all"):
            """Out-projection + LayerNorm for q row tile t. part="front"
            stops before the ScalarE rstd (so overlapping out_lns don't
            interleave table sets with the attention exps); "back" finishes."""
            if part == "back":
                ot, scr, mv, rstd = ln_pend.pop(t)
                nc.scalar.activation(rstd, mv[:, 1:2], AF.Abs_reciprocal_sqrt,
                                     bias=eps)
                for cc in range(VN):
                    cs = slice(cc * VC, (cc + 1) * VC)
                    nc.vector.tensor_scalar(
                        ot[:, cs], ot[:, cs], mv[:, 0:1], rstd,
                        ALU.subtract, ALU.mult)
                    nc.sync.dma_start(
                        out.rearrange("(t p) d -> p t d", p=P)[:, t, cs],
                        ot[:, cs])
                return
            FSUB = min(512, D)
            NSUB = D // FSUB
            ot = outp.tile([P, D], F32, tag="ot", name="ot")
            for nn in range(VN):
                # post-attention out-projs borrow the idle scores pool so
                # the tail double-buffers without contending with the
                # transpose finishers in mmp
                if t >= QS:
                    ps = scp.tile([P, 2, 512], F32, tag="sc",
                                  name="ps")[:, 0, :VC]
                else:
                    ps = mmp.tile([P, 512], F32, tag="mm", name="ps")[:, :VC]
                for k in range(ET):
                    nc.tensor.matmul(
                        ps, aot[:, k, t * P:(t + 1) * P],
                        wo[:, k, nn * VC:(nn + 1) * VC],
                        start=(k == 0), stop=(k == ET - 1),
                    )
                nc.vector.tensor_tensor(
                    ot[:, nn * VC:(nn + 1) * VC], ps,
                    brow["bo"][:, nn * VC:(nn + 1) * VC], ALU.add)
            scr = lnp.tile([P, NSUB * 6 + 3], F32, tag="scr", name="scr")
            stats = scr[:, 0:NSUB * 6].rearrange("p (s f) -> p s f", f=6)
            mv = scr[:, NSUB * 6:NSUB * 6 + 2]
            rstd = scr[:, NSUB * 6 + 2:NSUB * 6 + 3]
            otv = ot.rearrange("p (s f) -> p s f", f=FSUB)
            for sbi in range(NSUB):
                nc.vector.bn_stats(stats[:, sbi, :], otv[:, sbi, :])
            nc.vector.bn_aggr(mv, stats)
            if part == "front":
                ln_pend[t] = (ot, scr, mv, rstd)
                return
            nc.scalar.activation(rstd, mv[:, 1:2], AF.Abs_reciprocal_sqrt,
                                 bias=eps)
            # device emits the normalized rows; the ln_w/ln_b affine is a
            # pure elementwise epilogue applied host-side on the gathered
            # output. Chunked so each output DMA starts as soon as its
            # half is normalized.
            for cc in range(VN):
                cs = slice(cc * VC, (cc + 1) * VC)
                nc.vector.tensor_scalar(
                    ot[:, cs], ot[:, cs], mv[:, 0:1], rstd,
                    ALU.subtract, ALU.mult)
                nc.sync.dma_start(
                    out.rearrange("(t p) d -> p t d", p=P)[:, t, cs],
                    ot[:, cs])

        # --- emission schedule ---
        wo = wop.tile([P, ET, D], dtype, tag="wo")
        slots_left[0] = ET * QN * SJ
        TOT_SLOTS[0] = slots_left[0]
        qk_proj(0, pre0)
        wv0 = load_wv(0, nc.sync)
        v_proj(0, wv0, defer="urgent")
        qk_proj(1, pre1, defer=True)
        nc.gpsimd.dma_start(wo, ws["wo"].rearrange("(t p) d -> p t d", p=P))
        done_ln = [0]

        def queue_ln(upto, part="all"):
            while done_ln[0] < upto:
                t = done_ln[0]
                urgent_q.append(lambda t=t, part=part: out_ln(t, part))
                done_ln[0] += 1

        for m in range(ET):
            for n in range(QN):
                if m == ET - 1 and n == 1:
                    # first-half out-projs are ready: overlap them with
                    # the final attention block (ScalarE-free front part)
                    queue_ln(QS, "front")
                attention(m, n)
            # queue the remaining projections; the attention j-loops drain
            # them at a rate sized to last through all the blocks
            if m + 2 < ET:
                qk_proj(m + 2, defer=True)
            if m == 0:
                v_proj(1, load_wv(1), defer=True)
        if debug:
            nc.sync.dma_start(dbg["dq0"], qts[0])
            nc.sync.dma_start(dbg["dk0"], kts[0])
            nc.sync.dma_start(
                dbg["dv"], vt.rearrange("p a b c -> p (a b c)"))
        while tail_work:
            tail_work.pop(0)()
        # drain remaining finishers interleaved with the out_lns they gate
        while urgent_q:
            urgent_q.pop(0)()
            queue_ln(min(done_ln[0] + 1, TQ))
        for t in sorted(ln_pend):
            out_ln(t, "back")
        while fill_q:
            fill_q.pop(0)[1]()

        if debug:
            nc.sync.dma_start(dbg["daot"], aot.rearrange("p a b -> p (a b)"))
        # out-proj/LN remainder
        queue_ln(TQ)
        while urgent_q:
            urgent_q.pop(0)()

    nc.compile()
    return nc


# ---------------------------------------------------------------- host side

_CACHE = {}


def _get_nc(S, SQ, D, H):
    key = (S, SQ, D, H)
    if key not in _CACHE:
        _CACHE[key] = build_bass(S, SQ, D, H)
    return _CACHE[key]


F8NP = ml_dtypes.float8_e4m3


def _split8(a):
    """fp32 array -> (hi, lo) e4m3 planes with hi+lo ~= a."""
    hi = np.asarray(a, np.float32).astype(F8NP)
    lo = (np.asarray(a, np.float32) - hi.astype(np.float32)).astype(F8NP)
    return hi, lo


def make_in_maps(x, Wq, bq, Wk, bk, Wv, bv, Wo, bo, ln_w, ln_b, n_cores=8):
    """Shard full inputs into per-core input maps (batch x seq-half)."""
    B, S, D = x.shape
    halves = n_cores // B
    SQ = S // halves
    bf = ml_dtypes.bfloat16
    ET = D // 128
    P = 128

    def pack_qk(W):
        # base[m, p, t, d] = (W*32)[m*128+d, t*128+p]
        hi, lo = _split8(np.asarray(W, np.float32) * WSCALE)

        def arr(z):
            w4 = z.astype(np.float32).T.reshape(ET, P, ET, P)  # [t, p, m, d]
            return np.ascontiguousarray(w4.transpose(2, 1, 0, 3))  # [m,p,t,d]

        ah, al = arr(hi), arr(lo)
        full = np.stack([ah, ah], axis=3)  # [m, p, t, 2, d] (hi dup)
        return (full.reshape(ET, P, ET * 2 * P).astype(F8NP),
                np.ascontiguousarray(al).reshape(ET, P, ET * P).astype(F8NP))

    def pack_v(W):
        hi, lo = _split8(np.asarray(W, np.float32) * WSCALE)

        def arr(z):  # [p, t, dcol] = z.T[t*128+p, dcol]
            return np.ascontiguousarray(
                z.astype(np.float32).T.reshape(ET, P, D).transpose(1, 0, 2))

        ah, al = arr(hi), arr(lo)
        full = np.stack([ah, ah], axis=2)  # [p, t, 2, d] (hi dup)
        return (full.reshape(P, ET * 2 * D).astype(F8NP),
                al.reshape(P, ET * D).astype(F8NP))

    wq8, wqh8 = pack_qk(Wq)
    wk8, wkh8 = pack_qk(Wk)
    wv8, wvh8 = pack_v(Wv)
    common = {
        "wq8": wq8, "wqh8": wqh8, "wk8": wk8, "wkh8": wkh8,
        "wv8": wv8, "wvh8": wvh8,
        "wo": np.ascontiguousarray(np.asarray(Wo).T).astype(bf),
        "ident": np.eye(P, dtype=np.float32).astype(bf),
        "bq": np.asarray(bq, np.float32), "bk": np.asarray(bk, np.float32),
        "bv": (np.asarray(bv, np.float32) * WSCALE).astype(bf),
        "bo": np.asarray(bo, np.float32).astype(bf),
    }
    in_maps = []
    for c in range(n_cores):
        b, half = c // halves, c % halves
        xTb = np.asarray(x[b], np.float32).T  # [D, S]
        if half:
            xTb = np.roll(xTb, -half * SQ, axis=1)
        hi, lo = _split8(xTb)
        hi = hi.reshape(ET, P, S)
        lo = lo.reshape(ET, P, S)
        x8 = np.empty((P, ET, 2, S), F8NP)
        x8[:, :, 0] = hi.transpose(1, 0, 2)
        x8[:, :, 1] = lo.transpose(1, 0, 2)
        in_maps.append({"x8": np.ascontiguousarray(x8), **common})
    return in_maps, SQ


def kernel(x, Wq, bq, Wk, bk, Wv, bv, Wo, bo, ln_w, ln_b, _trace=False):
    x = np.asarray(x)
    B, S, D = x.shape
    n_cores = 8
    in_maps, SQ = make_in_maps(x, Wq, bq, Wk, bk, Wv, bv, Wo, bo, ln_w, ln_b,
                               n_cores)
    nc = _get_nc(S, SQ, D, 16)
    res = run_bass_kernel_spmd(nc, in_maps, list(range(n_cores)), trace=_trace)
    out = np.empty((B, S, D), np.float32)
    halves = n_cores // B
    for c in range(n_cores):
        b, half = c // halves, c % halves
        out[b, half * SQ:(half + 1) * SQ] = res.results[c]["out"]
    out *= np.asarray(ln_w, np.float32)
    out += np.asarray(ln_b, np.float32)
    kernel.last_result = res
    return out


if __name__ == "__main__":
    nc = build_bass(2048, 1024, 1024, 16)
    print("built ok")
